# revision 1
# baseline (speedup 1.0000x reference)
"""ClusterNet (vq_codebook) kernel for 8x Trainium2 NeuronCores (Bass/Tile).

Reference math (ALPHA = 1):
    d2   = |z - c|^2                     z: (8192, 2048)  c: (512, 2048)
    Qun  = (1 + sqrt(d2))^-1
    Q    = Qun / rowsum(Qun)
    P    = (Q^2 / colsum(Q)) / rowsum(Q^2 / colsum(Q))
    out  = stack([Q, P])                 (2, 8192, 512) float32

Distribution: data-parallel over the batch — 1024 rows per core, centroids
replicated. Cross-core communication is an AllGather of the per-core
column-sum of Q (512 floats); each core then reduces + broadcasts the 8
partials with a single K=8 matmul.

Per-core pipeline (8 m-tiles of 128 rows):
  PE   : PSUM accumulates d2 - 1 per tile: one K=2 f32r affine matmul
         (rows csq-1 / zsq against ones) + 8 fp8e4 DoubleRow matmuls
         (K=256 each) carrying the -2*z.c cross term. Tiles 0-3 stream
         k-outer so matmuls start as soon as the first input chunk lands;
         tiles 4-7 run m-outer so their results pipeline into the tail.
  ACT  : sim = Sqrt(psum + 1) = sqrt(d2).
  DVE  : r = 1/(d2-1) (approx_fast); qun = (sim - 1) * r  [since
         1/(1+s) = (s-1)/(d2-1)] with fused row-sum; rq = 1/rowsum.
  ACT  : Q = qun * rq (per-partition scale), written bf16.
  PE   : per-tile weighted matvec (lhsT=rq, rhs=qun, f32r) accumulates the
         local colsum of Q in PSUM.
  CC   : AllGather of the [1,512] partial (a dummy warm-up AllGather is
         triggered at kernel start so ncfw is awake and launch skew is
         absorbed off the critical path).
  PE   : ones[8,128]^T @ gathered[8,512] = colsum broadcast to 128
         partitions in one matmul; DVE reciprocal -> 1/s.
  DVE  : W = qun^2 * (1/s) with fused row-accumulate (qun^2 via ACT Square
         scheduled under the collective); P = W * (1/rowsumW) on ACT, bf16.

Host prepares fp8 transposed operands and exact f32 squared norms; Q/P
come back bf16 and are upcast to f32 on the host.
"""

import sys

import numpy as np

if "/opt/trn_rl_repo" not in sys.path:
    sys.path.insert(0, "/opt/trn_rl_repo")

import ml_dtypes

import concourse.bacc as bacc
import concourse.mybir as mybir
import concourse.tile as tile
from concourse.bass_utils import run_bass_kernel_spmd

F8 = ml_dtypes.float8_e4m3
BF16 = ml_dtypes.bfloat16

N_CORES = 8
BS, NH, NCL = 8192, 2048, 512
B_CORE = BS // N_CORES          # 1024 rows per core
M_TILES = B_CORE // 128         # 8
G = NH // 256                   # 8 DoubleRow groups (256 contraction rows each)

_nc_cache = None


def _build_nc():
    F = mybir.ActivationFunctionType
    A = mybir.AluOpType
    f32 = mybir.dt.float32
    f32r = mybir.dt.float32r
    bf16 = mybir.dt.bfloat16
    f8 = mybir.dt.float8e4
    DR = mybir.MatmulPerfMode.DoubleRow

    nc = bacc.Bacc("TRN2", target_bir_lowering=False, debug=False,
                   num_devices=N_CORES)
    zt_d = nc.dram_tensor("zt", [G, 128, 2 * B_CORE], f8, kind="ExternalInput")
    ct_d = nc.dram_tensor("ct", [G, 128, 2 * NCL], f8, kind="ExternalInput")
    affl_d = nc.dram_tensor("affl", [2, B_CORE], f32r, kind="ExternalInput")
    affr_d = nc.dram_tensor("affr", [2, NCL], f32r, kind="ExternalInput")
    ones8_d = nc.dram_tensor("ones8", [8, 128], f32r, kind="ExternalInput")

    q_out = nc.dram_tensor("q", [B_CORE, NCL], bf16, kind="ExternalOutput")
    p_out = nc.dram_tensor("p", [B_CORE, NCL], bf16, kind="ExternalOutput")

    groups = [list(range(N_CORES))]

    with tile.TileContext(nc) as tc:
        with (
            tc.tile_pool(name="zin", bufs=1) as zin,
            tc.tile_pool(name="cin", bufs=1) as cin,
            tc.tile_pool(name="work", bufs=1) as work,
            tc.tile_pool(name="small", bufs=1) as small,
            tc.tile_pool(name="qout", bufs=3) as qout,
            tc.tile_pool(name="pout", bufs=3) as pout,
            tc.tile_pool(name="psum", bufs=6, space="PSUM") as psum,
            tc.tile_pool(name="cpsum", bufs=1, space="PSUM") as cpsum,
            tc.tile_pool(name="dram", bufs=1, space="DRAM") as dram,
        ):
            # --- input DMA: one transfer per DoubleRow group so the PE can
            # start as soon as group 0 lands (zt on sync, ct on vector queue)
            zt = zin.tile([128, G, 2, B_CORE], f8, tag="zt")
            ct = cin.tile([128, G, 2, NCL], f8, tag="ct")
            affl = small.tile([2, B_CORE], f32r, tag="affl")
            nc.scalar.dma_start(out=affl, in_=affl_d.ap())
            affr = small.tile([2, NCL], f32r, tag="affr")
            nc.scalar.dma_start(out=affr, in_=affr_d.ap())

            for g in range(G):
                nc.scalar.dma_start(
                    out=ct[:, g], in_=ct_d.ap()[g].rearrange(
                        "p (k n) -> p k n", k=2))
                nc.sync.dma_start(
                    out=zt[:, g], in_=zt_d.ap()[g].rearrange(
                        "p (k m) -> p k m", k=2))

            # --- workspaces
            sim_all = work.tile([128, M_TILES, NCL], f32, tag="sim")
            r_all = work.tile([128, M_TILES, NCL], f32, tag="r")
            qun_all = work.tile([128, M_TILES, NCL], f32, tag="qun")
            q2_all = work.tile([128, M_TILES, NCL], f32, tag="q2")
            w_all = work.tile([128, M_TILES, NCL], f32, tag="w")
            sq_all = small.tile([128, M_TILES], f32, tag="sq")
            rq_all = small.tile([128, M_TILES], f32, tag="rq")
            ws_all = small.tile([128, M_TILES], f32, tag="ws")
            rw_all = small.tile([128, M_TILES], f32, tag="rw")
            ones8 = small.tile([8, 128], f32r, tag="ones8")
            nc.scalar.dma_start(out=ones8, in_=ones8_d.ap())
            ones_bf = small.tile([128, 1], bf16, tag="onesbf")
            nc.vector.memset(ones_bf, 1.0)
            q_all = work.tile([128, M_TILES, NCL], bf16, tag="qall")
            cs_sb = small.tile([1, NCL], f32, tag="cssb")
            ag_sb = small.tile([8, NCL], f32r, tag="agsb")
            rs_inv = small.tile([128, NCL], f32, tag="rsinv")

            warm_in = dram.tile([1, 8], f32)
            warm_out = dram.tile([8, 8], f32, addr_space="Shared")
            cc_in = dram.tile([1, NCL], f32)
            cc_out = dram.tile([8, NCL], f32, addr_space="Shared")

            # --- warm-up collective: pays the ~50us ncfw cold-start early,
            # in the background, so the real AllGather below runs on warm
            # ncfw. The payload is garbage (uninitialized DRAM) by design.
            nc.gpsimd.collective_compute(
                "AllGather", A.bypass, replica_groups=groups,
                ins=[warm_in.opt()], outs=[warm_out.opt()],
            )

            ps = [None] * M_TILES

            def mm_tile(m):
                ps[m] = psum.tile([128, NCL], f32, name=f"ps{m}", tag="mm")

            def mm_group(m, g):
                ms = slice(m * 128, (m + 1) * 128)
                nc.tensor.matmul(
                    ps[m], lhsT=zt[:, g, :, ms], rhs=ct[:, g],
                    start=(g == 0), stop=False, perf_mode=DR)

            def mm_affine(m):
                ms = slice(m * 128, (m + 1) * 128)
                nc.tensor.matmul(
                    ps[m], lhsT=affl[:, ms], rhs=affr[:, :],
                    start=False, stop=True)

            def dve_tail(m):
                sim = sim_all[:, m, :]
                r = r_all[:, m, :]
                qun = qun_all[:, m, :]
                nc.vector.reciprocal_approx_fast(out=r, in_=ps[m][:, :])
                nc.vector.scalar_tensor_tensor(
                    out=qun, in0=sim, scalar=1.0, in1=r,
                    op0=A.subtract, op1=A.mult,
                    accum_out=sq_all[:, m:m + 1])
                nc.vector.reciprocal(rq_all[:, m:m + 1], sq_all[:, m:m + 1])

            def act_sqrt(m):
                nc.scalar.activation(sim_all[:, m, :], ps[m][:, :],
                                     F.Sqrt, bias=1.0)

            def q_store(m):
                nc.scalar.activation(q_all[:, m, :], qun_all[:, m, :], F.Copy,
                                     scale=rq_all[:, m:m + 1])
                nc.sync.dma_start(
                    out=q_out.ap()[m * 128:(m + 1) * 128, :],
                    in_=q_all[:, m, :])

            cps = cpsum.tile([1, NCL], f32, tag="cs")

            def matvec(m, start, stop):
                nc.tensor.matmul(
                    cps, lhsT=ones_bf[:, :], rhs=q_all[:, m, :],
                    start=start, stop=stop)

            # wave A: tiles 0-3, k-outer (stream groups as they arrive)
            for m in range(4):
                mm_tile(m)
            for g in range(G):
                for m in range(4):
                    mm_group(m, g)
            for m in range(4):
                mm_affine(m)
            for m in range(4):
                act_sqrt(m)
            for m in range(4):
                dve_tail(m)
            for m in range(4):
                q_store(m)

            # wave B: tiles 4-7, m-outer; wave-A matvecs ride between tiles
            for m in range(4, M_TILES):
                mm_tile(m)
                for g in range(G):
                    mm_group(m, g)
                mm_affine(m)
                matvec(m - 4, start=(m == 4), stop=False)
                act_sqrt(m)
                dve_tail(m)
                q_store(m)
            for m in range(4, M_TILES):
                matvec(m, start=False, stop=(m == M_TILES - 1))

            # squares run under the collective window
            for m in range(M_TILES):
                nc.scalar.activation(q2_all[:, m, :], qun_all[:, m, :],
                                     F.Square)

            # local colsum -> AllGather -> sum+broadcast via one K=8 matmul
            nc.vector.tensor_copy(cs_sb, cps)
            nc.sync.dma_start(out=cc_in[:, :], in_=cs_sb)
            nc.gpsimd.collective_compute(
                "AllGather", A.bypass, replica_groups=groups,
                ins=[cc_in.opt()], outs=[cc_out.opt()],
            )
            nc.sync.dma_start(out=ag_sb, in_=cc_out[:, :].bitcast(f32r))
            bps = cpsum.tile([128, NCL], f32, tag="bps")
            nc.tensor.matmul(bps, lhsT=ones8[:, :], rhs=ag_sb[:, :],
                             start=True, stop=True)
            nc.vector.reciprocal_approx_fast(out=rs_inv, in_=bps[:, :])

            # P phase
            for m in range(M_TILES):
                nc.vector.scalar_tensor_tensor(
                    out=w_all[:, m, :], in0=q2_all[:, m, :],
                    scalar=0.0, in1=rs_inv,
                    op0=A.bypass, op1=A.mult,
                    accum_out=ws_all[:, m:m + 1])
                nc.vector.reciprocal(rw_all[:, m:m + 1], ws_all[:, m:m + 1])
                pt = pout.tile([128, NCL], bf16, tag="pt")
                if m % 2 == 0:
                    nc.scalar.activation(pt, w_all[:, m, :], F.Copy,
                                         scale=rw_all[:, m:m + 1])
                else:
                    nc.vector.tensor_scalar_mul(pt, w_all[:, m, :],
                                                rw_all[:, m:m + 1])
                nc.sync.dma_start(
                    out=p_out.ap()[m * 128:(m + 1) * 128, :], in_=pt)
    nc.compile()
    return nc


def _get_nc():
    global _nc_cache
    if _nc_cache is None:
        _nc_cache = _build_nc()
    return _nc_cache


def _prep_inputs(z, centroids):
    z = np.asarray(z, dtype=np.float32)
    c = np.asarray(centroids, dtype=np.float32)

    # fp8 cross-term operands; contraction row h = 256g + 128ko + ki
    z8 = z.astype(F8)                                   # (8192, 2048)
    c8m2 = (-2.0 * c.astype(F8).astype(np.float32)).astype(F8)
    ct_full = np.ascontiguousarray(
        c8m2.T.reshape(G, 2, 128, NCL).transpose(0, 2, 1, 3)
    ).reshape(G, 128, 2 * NCL)

    csq = np.sum(c.astype(np.float64) ** 2, axis=1)     # (512,)
    affr = np.empty((2, NCL), dtype=np.float32)
    affr[0] = (csq - 1.0).astype(np.float32)
    affr[1] = 1.0

    zsq = np.sum(z.astype(np.float64) ** 2, axis=1)     # (8192,)

    in_maps = []
    for core in range(N_CORES):
        s = slice(core * B_CORE, (core + 1) * B_CORE)
        zt_core = np.ascontiguousarray(
            z8[s].T.reshape(G, 2, 128, B_CORE).transpose(0, 2, 1, 3)
        ).reshape(G, 128, 2 * B_CORE)
        affl = np.empty((2, B_CORE), dtype=np.float32)
        affl[0] = 1.0
        affl[1] = zsq[s].astype(np.float32)
        in_maps.append({"zt": zt_core, "ct": ct_full,
                        "affl": affl, "affr": affr,
                        "ones8": np.ones((8, 128), dtype=np.float32)})
    return in_maps


def run(z, centroids, trace=False, trace_cores=None):
    """Run on the 8 NeuronCores. Returns (out, BassKernelResults)."""
    nc = _get_nc()
    in_maps = _prep_inputs(z, centroids)
    res = run_bass_kernel_spmd(
        nc, in_maps, list(range(N_CORES)),
        trace=trace, trace_cores=trace_cores,
    )
    q = np.concatenate([np.asarray(res.results[c]["q"], dtype=np.float32)
                        for c in range(N_CORES)], axis=0)
    p = np.concatenate([np.asarray(res.results[c]["p"], dtype=np.float32)
                        for c in range(N_CORES)], axis=0)
    out = np.stack([q, p])
    return out, res


def kernel(z, centroids):
    out, _ = run(z, centroids)
    return out

